# revision 1
# baseline (speedup 1.0000x reference)
"""MultiHeadSelfAttention TRN2 kernel — head-tensor-parallel over 8 NeuronCores.

Reference semantics (note the quirk: softmax over the QUERY axis):
    Q = x @ Wq[h].T + bq[h]            [B,S,D] per head
    K = x @ Wk[h].T + bk[h]
    V = x @ Wv[h].T + bv[h]
    scores[s,t] = (Q[s]·K[t]) / sqrt(D)
    attn = softmax over s (query axis)  -> attn[s,t] = exp(sc[s,t]) / sum_s' exp(sc[s',t])
    Z[s] = sum_t attn[s,t] V[t]
    out = concat_heads(Z) @ Wo.T + bo

Sharding: head h -> core h. Each core computes its head's partial output
projection out_h = Z_h @ Wo[:, h*D:(h+1)*D].T ; host sums the 8 partials
(the all-reduce after W_o, done on host during the gather) ; bo is folded
into core 0's partial.

Layout strategy (everything transposed so the quirky softmax normalization
axis 's' lands on the free dimension):
    xT   [d, s]   QT = WqT.T @ xT   [e, s]      (bf16)
    KT   [e, t],  V [t, e]                      (bf16)
    scoresT[t, s] = KT.T @ QT  (bf16 matmul) -> exp with ACT accum_out
                                                => denom[t] for free
    V'[t,:] = V[t,:] * (C / denom[t])           (stored fp8e4)
    ZT[e, s] = V'.T @ PT   via fp8e4 DoubleRow matmuls (two t-blocks of
               contraction per instruction, 2x PE rate)   (bf16 out)
    outT[o, s] = WoHT.T @ ZT  (bf16 matmul, Wo pre-scaled by 1/C on host)

fp8 scaling: PT = exp(score - ln PBIAS) keeps values below the TRN e4m3
max of 240 (256 encodes Inf).  V' = (C*V) / dn with dn = denom/PBIAS, so
ZT = C * sum_t attn*V and 1/C is folded into Wo host-side.

Schedule: ONE global stream of 128 score slices (4 batches x 2 superblocks
x 16 [128,1024] psum slices).  The exp stream on ACT (~1.1us/slice incl.
accum read) is slower than the PE's score production (~0.85us/slice), so
every other piece of PE work — V projection chunks, fp8 ZT quarters, the
output projection, and the NEXT batch's Q/K projection chunks — is emitted
as ~0.4-0.6us "filler" items popped from a global queue between score
slices.  Score slices own the 6 "acc" PSUM banks (3 x [128,1024]); all
fillers run on the 2 rotating "z" banks, so filler PSUM pressure never
blocks the ACT pipeline.  At each batch boundary the next batch's qt/kt
sh0 chunks are staggered into the last 4 slices so the scores stream never
pauses.  Projection drains ride DVE; ACT does only exp(+accum).

PSUM budget: acc [128,1024] x3 bufs = 6 banks (score slices; also the
batch-0 head projections, emitted before any score slice exists), z0/z1
[128,512] = 2 rotating banks for every filler item.
"""

import numpy as np
import ml_dtypes

import concourse.bass as bass
import concourse.mybir as mybir
import concourse.tile as tile
from concourse import bacc
from concourse.bass_utils import run_bass_kernel_spmd

B, S, D, H = 4, 2048, 256, 8
N_CORES = 8
P = 128          # partitions
NDB = D // P     # 2 d-blocks (contraction blocks for projections)
NTB = S // P     # 16 key/t blocks
SC = 512         # matmul moving-dim chunk == z psum tile width
NSC = S // SC    # 4 s chunks
SH = 1024        # s-half (scores psum granularity)
NSH = S // SH    # 2 s halves
G = 8            # t-blocks per superblock
NSUP = NTB // G  # 2 superblocks

f32 = mybir.dt.float32
bf16 = mybir.dt.bfloat16
f8 = mybir.dt.float8e4
DR = mybir.MatmulPerfMode.DoubleRow
EXP = mybir.ActivationFunctionType.Exp

# fp8 scaling constants (see module docstring)
PBIAS = 8.0
VC = 1024.0


def _build():
    nc = bacc.Bacc(target_bir_lowering=False)

    xT = nc.dram_tensor("xT", [B, D, S], bf16, kind="ExternalInput")
    wqT = nc.dram_tensor("wqT", [D, D], bf16, kind="ExternalInput")  # (Wq/sqrt(D)).T
    wkT = nc.dram_tensor("wkT", [D, D], bf16, kind="ExternalInput")
    wvT = nc.dram_tensor("wvT", [D, D], bf16, kind="ExternalInput")  # * VC
    woT = nc.dram_tensor("woT", [D, D], bf16, kind="ExternalInput")  # / VC
    bqc = nc.dram_tensor("bqc", [D, 1], f32, kind="ExternalInput")
    bkc = nc.dram_tensor("bkc", [D, 1], f32, kind="ExternalInput")
    bvb = nc.dram_tensor("bvb", [P, 2 * D], f32, kind="ExternalInput")  # bv*VC x2
    boc = nc.dram_tensor("boc", [D, 1], f32, kind="ExternalInput")  # bo (core0) / 0
    ebc = nc.dram_tensor("ebc", [P, 1], f32, kind="ExternalInput")  # -ln(PBIAS)
    outT = nc.dram_tensor("outT", [B, D, S], f32, kind="ExternalOutput")

    with tile.TileContext(nc) as tc:
        with (
            tc.tile_pool(name="const", bufs=1) as cpool,
            tc.tile_pool(name="qtp", bufs=2) as qtpool,
            tc.tile_pool(name="xtp", bufs=2) as xtpool,
            tc.tile_pool(name="ktp", bufs=2) as ktpool,
            tc.tile_pool(name="vap", bufs=2) as vpool,
            tc.tile_pool(name="ztp", bufs=2) as zpool,
            tc.tile_pool(name="pt", bufs=2) as ppool,
            tc.tile_pool(name="small", bufs=2) as spool,
            tc.tile_pool(name="outp", bufs=4) as opool,
            tc.tile_pool(name="ps_a", bufs=3, space="PSUM") as psa,
            tc.tile_pool(name="ps_z", bufs=1, space="PSUM") as psz,
        ):
            # ---- constants ----
            # Startup critical path: wq/wk (pool head) + xt sh0 gate the
            # first projections (every DMA pays a fixed ~900ns completion->
            # semaphore latency, so the heads of the sync/pool queues bound
            # the kernel start); biases ride the scalar queue behind the act
            # table load; wv/wo/bo land later.
            wq_t = cpool.tile([P, NDB, D], bf16, tag="wq")
            wk_t = cpool.tile([P, NDB, D], bf16, tag="wk")
            wv_t = cpool.tile([P, NDB, D], bf16, tag="wv")
            wo_t = cpool.tile([P, NDB, D], bf16, tag="wo")
            bq_t = cpool.tile([P, NDB, 1], f32, tag="bq")
            bk_t = cpool.tile([P, NDB, 1], f32, tag="bk")
            bo_t = cpool.tile([P, NDB, 1], f32, tag="bo")
            bvb_t = cpool.tile([P, 2 * D], f32, tag="bvb")
            eb_t = cpool.tile([P, 1], f32, tag="ebc")
            nc.gpsimd.dma_start(
                out=wk_t[:], in_=wkT.rearrange("(n p) e -> p n e", p=P)
            )
            nc.gpsimd.dma_start(
                out=wq_t[:], in_=wqT.rearrange("(n p) e -> p n e", p=P)
            )

            def emit_late_consts():
                nc.gpsimd.dma_start(out=bvb_t[:], in_=bvb[:])
                for w_t, w_d in ((wv_t, wvT), (wo_t, woT)):
                    nc.gpsimd.dma_start(
                        out=w_t[:], in_=w_d.rearrange("(n p) e -> p n e", p=P)
                    )
                nc.gpsimd.dma_start(
                    out=bo_t[:], in_=boc.rearrange("(n p) o -> p n o", p=P)
                )

            def load_xt(b, half=None, first=False):
                """Allocate + DMA xt for batch b as a list of (s0, size, tile)
                pieces (the framework's DMA waits are tile-granular, so a
                consumer only waits for the pieces it touches), striped over
                two queues.  half selects one half for staged batch-0 loading;
                batch 0's sh0 half is split into QUARTER tiles so the very
                first projection chunk waits only for the first 512 columns."""
                xT_r = xT[b].rearrange("(n p) s -> p n s", p=P)

                def one(h, engs):
                    t = xtpool.tile([P, NDB, SH], bf16, tag=f"xt{h}", name="xt")
                    for i, eng in enumerate(engs):
                        sq = 2 * h + i
                        eng.dma_start(
                            out=t[:, :, bass.ts(i, SC)],
                            in_=xT_r[:, :, bass.ts(sq, SC)],
                        )
                    return t

                def quarter(sq, eng, tag):
                    t = xtpool.tile([P, NDB, SC], bf16, tag=tag, name="xtq")
                    eng.dma_start(out=t[:], in_=xT_r[:, :, bass.ts(sq, SC)])
                    return t

                if half == 0:
                    return [
                        (0, SC, quarter(0, nc.sync, "xq0")),
                        (SC, SC, quarter(1, nc.gpsimd, "xq1")),
                    ]
                if half == 1:
                    return [(SH, SH, one(1, (nc.sync, nc.scalar)))]
                return [
                    (0, SH, one(0, (nc.sync, nc.gpsimd))),
                    (SH, SH, one(1, (nc.sync, nc.gpsimd))),
                ]

            def xsl(xt2, db, start, size):
                """Slice the split xt as if it were one [P, NDB, S] tile."""
                for s0, sz, t in xt2:
                    if s0 <= start < s0 + sz:
                        return t[:, db, bass.ds(start - s0, size)]
                raise AssertionError(start)

            zctr = [0]

            def ztag():
                zctr[0] += 1
                return f"z{zctr[0] % 2}"

            def emit_proj_chunk(xt2, dst, w, bias, eb, sc2, on_acc=False):
                """One [128,512] psum chunk of a Q/K projection; DVE drain
                (+bias) into the bf16 destination.  on_acc borrows the (idle
                at startup) acc banks instead of the 2-deep z rotation."""
                if on_acc:
                    psj = psa.tile([P, SC], f32, tag="acc", name="psj")
                else:
                    psj = psz.tile([P, SC], f32, tag=ztag(), name="psj")
                for db in range(NDB):
                    nc.tensor.matmul(
                        psj[:],
                        w[:, db, bass.ts(eb, P)],
                        xsl(xt2, db, sc2 * SC, SC),
                        start=(db == 0),
                        stop=(db == NDB - 1),
                    )
                nc.vector.tensor_scalar_add(
                    dst[:, eb, bass.ds(sc2 * SC, SC)], psj[:], bias[:, eb, :]
                )

            def emit_v_chunk(xt2, v_all, c):
                """V projection for t-blocks 2c, 2c+1 -> v_all (+bias)."""
                psv = psz.tile([P, 2 * D], f32, tag=ztag(), name="psv")
                for k in range(2):
                    tb = 2 * c + k
                    for db in range(NDB):
                        nc.tensor.matmul(
                            psv[:, bass.ts(k, D)],
                            xsl(xt2, db, tb * P, P),
                            wv_t[:, db, :],
                            start=(db == 0),
                            stop=(db == NDB - 1),
                        )
                nc.vector.tensor_add(
                    v_all[:, bass.ds(2 * c, 2), :],
                    psv[:].rearrange("p (g e) -> p g e", g=2),
                    bvb_t[:].rearrange("p (g e) -> p g e", g=2),
                )

            def emit_zt_q(zt, pt, vp, g, sq, eh, on_acc=False):
                """One ZT quarter (sq, eh) of superblock g — fp8 DoubleRow.
                In the tail the (by then idle) acc banks deepen the rotation."""
                if on_acc:
                    psz_t = psa.tile([P, SC], f32, tag="acc", name="psz_t")
                else:
                    psz_t = psz.tile([P, SC], f32, tag=ztag(), name="psz_t")
                ssl = bass.ts(sq, SC)
                for m in range(G // 2):
                    nc.tensor.matmul(
                        psz_t[:],
                        vp[:, 2 * m : 2 * m + 2, bass.ts(eh, P)],
                        pt[:, 2 * m : 2 * m + 2, ssl],
                        start=(m == 0),
                        stop=(m == G // 2 - 1),
                        perf_mode=DR,
                    )
                zsl = zt[:, eh, ssl]
                if g == 0:
                    nc.vector.tensor_copy(zsl, psz_t[:])
                else:
                    nc.vector.tensor_add(zsl, zsl, psz_t[:])

            def emit_op_item(b, zt, ob, sq, on_act=False):
                """One 512-wide chunk of the output projection of batch b.
                In the tail the drain rides the (by then idle) ACT engine so
                DVE keeps up with the ZT drains."""
                osb = opool.tile([P, SC], f32, tag="osb", name="osb")
                pso = psz.tile([P, SC], f32, tag=ztag(), name="pso")
                ssl = bass.ts(sq, SC)
                for eh in range(NDB):
                    nc.tensor.matmul(
                        pso[:],
                        wo_t[:, eh, bass.ts(ob, P)],
                        zt[:, eh, ssl],
                        start=(eh == 0),
                        stop=(eh == NDB - 1),
                    )
                if on_act:
                    nc.scalar.activation(
                        osb[:], pso[:], mybir.ActivationFunctionType.Identity,
                        bias=bo_t[:, ob, :],
                    )
                else:
                    nc.vector.tensor_scalar_add(osb[:], pso[:], bo_t[:, ob, :])
                eng = nc.sync if (ob + sq) % 2 == 0 else nc.gpsimd
                eng.dma_start(out=outT[b, bass.ts(ob, P), ssl], in_=osb[:])

            def emit_scores_slice(qt, kt, pt, dnp, g, j, sh):
                """scores + biased exp for t-block g*G+j, query half sh."""
                tb = g * G + j
                pssc = psa.tile([P, SH], f32, tag="acc", name="pssc")
                for sc in range(SH // SC):
                    ssl = bass.ds(sh * SH + sc * SC, SC)
                    psl = bass.ts(sc, SC)
                    for eb in range(NDB):
                        nc.tensor.matmul(
                            pssc[:, psl],
                            kt[:, eb, bass.ts(tb, P)],
                            qt[:, eb, ssl],
                            start=(eb == 0),
                            stop=(eb == NDB - 1),
                        )
                nc.scalar.activation(
                    pt[:, j, bass.ts(sh, SH)],
                    pssc[:],
                    EXP,
                    bias=eb_t[:],
                    accum_out=dnp[:, j, sh : sh + 1],
                )

            def emit_norm_half(v_all, vp, dnp, g, h):
                """denominators -> reciprocal -> fp8 V' for half a superblock."""
                hg = G // 2
                dn = spool.tile([P, hg], f32, tag=f"dn{h}", name="dn")
                rc = spool.tile([P, hg], f32, tag=f"rc{h}", name="rc")
                jsl = bass.ds(h * hg, hg)
                nc.vector.tensor_add(dn[:], dnp[:, jsl, 0], dnp[:, jsl, 1])
                nc.vector.reciprocal(rc[:], dn[:])
                for j in range(hg):
                    ja = h * hg + j
                    nc.vector.tensor_scalar_mul(
                        vp[:, ja, :], v_all[:, g * G + ja, :], rc[:, j : j + 1]
                    )

            # ---- global filler queue ----
            fillq = []
            fq = [0]

            def backlog():
                return len(fillq) - fq[0]

            def pop_fill():
                n = 2 if backlog() > 15 else 1
                for _ in range(n):
                    if fq[0] < len(fillq):
                        fillq[fq[0]]()
                        fq[0] += 1

            # ---- batch-0 head.  Order matters: the framework's DMA waits
            # are coarse (a consumer waits for everything already emitted on
            # the queues it touches), so the kt/qt sh0 matmuls go FIRST —
            # gated only by wq/wk/xt-sh0 — and every other startup DMA (xt
            # sh1, biases, late consts) is emitted after them, before the
            # psum drains that need the biases. ----
            nc.sync.dma_start(
                out=bk_t[:], in_=bkc.rearrange("(n p) o -> p n o", p=P)
            )
            xt0 = load_xt(0, half=0)
            kt = ktpool.tile([P, NDB, S], bf16, tag="kt", name="kt")
            qt = qtpool.tile([P, NDB, S], bf16, tag="qt", name="qt")
            v_all = vpool.tile([P, NTB, D], bf16, tag="v", name="v_all")
            nc.sync.dma_start(
                out=bq_t[:], in_=bqc.rearrange("(n p) o -> p n o", p=P)
            )
            nc.scalar.dma_start(out=eb_t[:], in_=ebc[:])
            xt1 = load_xt(0, half=1)
            xt = xt0 + xt1
            for eb in range(NDB):
                emit_proj_chunk(xt, kt, wk_t, bk_t, eb, 0, on_acc=True)
            for sc2 in (0, 1):
                for eb in range(NDB):
                    emit_proj_chunk(xt, qt, wq_t, bq_t, eb, sc2, on_acc=True)
            emit_late_consts()
            fillq += [
                lambda eb=eb, xx=xt, kk=kt: emit_proj_chunk(xx, kk, wk_t, bk_t, eb, 1)
                for eb in range(NDB)
            ]
            fillq += [
                lambda eb=eb, sc2=sc2, xx=xt, qq=qt: emit_proj_chunk(
                    xx, qq, wq_t, bq_t, eb, sc2
                )
                for sc2 in (2, 3)
                for eb in range(NDB)
            ]
            fillq += [
                lambda eb=eb, sc2=sc2, xx=xt, kk=kt: emit_proj_chunk(
                    xx, kk, wk_t, bk_t, eb, sc2
                )
                for sc2 in (2, 3)
                for eb in range(NDB)
            ]
            fillq += [
                lambda c=c, xx=xt, vv=v_all: emit_v_chunk(xx, vv, c) for c in range(8)
            ]

            prev = None  # (batch, zt, pt_g1, vp_g1)
            for b in range(B):
                zt = zpool.tile([P, NDB, S], bf16, tag="zt", name="zt")
                pt0 = ppool.tile([P, G, S], f8, tag="pt", name="pt0")
                vp0 = ppool.tile([P, G, D], f8, tag="vp", name="vp0")
                dnp0 = spool.tile([P, G, NSH], f32, tag="dnp", name="dnp0")
                pt1 = ppool.tile([P, G, S], f8, tag="pt", name="pt1")
                vp1 = ppool.tile([P, G, D], f8, tag="vp", name="vp1")
                dnp1 = spool.tile([P, G, NSH], f32, tag="dnp", name="dnp1")

                # previous batch's g1 ZT quarters + output projection become
                # fillers of this batch's g0 window (sq-major; each op chunk
                # follows the ZT adds it needs).
                rsv = []
                if prev is not None:
                    pb, pzt, ppt1, pvp1 = prev
                    last = b + 1 == B
                    for sq in range(NSC):
                        fillq += [
                            lambda sq=sq, eh=eh: emit_zt_q(pzt, ppt1, pvp1, 1, sq, eh)
                            for eh in range(NDB)
                        ]
                        if sq >= 1:
                            ops = [
                                lambda ob=ob, sq=sq: emit_op_item(pb, pzt, ob, sq - 1)
                                for ob in range(NDB)
                            ]
                            if last and sq >= 2:
                                rsv += ops
                            else:
                                fillq += ops
                    ops = [
                        lambda ob=ob: emit_op_item(pb, pzt, ob, NSC - 1)
                        for ob in range(NDB)
                    ]
                    if last:
                        rsv += ops
                    else:
                        fillq += ops

                # ---- g0 scores ----
                for sh in range(NSH):
                    for j in range(G):
                        emit_scores_slice(qt, kt, pt0, dnp0, 0, j, sh)
                        if sh == 1 and j == 3:
                            emit_norm_half(v_all, vp0, dnp0, 0, 0)
                        pop_fill()
                emit_norm_half(v_all, vp0, dnp0, 0, 1)

                # this batch's g0 ZT quarters (vp0 just landed).  For the
                # last batch they are held back and spent in the final g1
                # slices, where the filler queue would otherwise run dry and
                # the PE would idle while ACT drains its exp backlog.
                g0q_items = [
                    lambda sq=sq, eh=eh, z=zt, p=pt0, v=vp0: emit_zt_q(
                        z, p, v, 0, sq, eh
                    )
                    for sq in range(NSC)
                    for eh in range(NDB)
                ]
                if b + 1 < B:
                    fillq += g0q_items
                else:
                    rsv = g0q_items + rsv

                # ---- g1 scores; next batch's projections stagger into the
                # last 4 slices so the score stream never pauses ----
                nxt_xt = nxt_kt = nxt_qt = nxt_v = None
                for sh in range(NSH):
                    for j in range(G):
                        emit_scores_slice(qt, kt, pt1, dnp1, 1, j, sh)
                        if sh == 0 and j == 1 and b + 1 < B:
                            # xt(b+1) DMA queues behind this window's output
                            # DMAs; V/K-sh1 of b+1 join the filler queue.
                            nxt_xt = load_xt(b + 1)
                            nxt_kt = ktpool.tile([P, NDB, S], bf16, tag="kt", name="kt")
                            nxt_qt = qtpool.tile([P, NDB, S], bf16, tag="qt", name="qt")
                            nxt_v = vpool.tile([P, NTB, D], bf16, tag="v", name="v_all")
                            fillq += [
                                lambda c=c, xx=nxt_xt, vv=nxt_v: emit_v_chunk(xx, vv, c)
                                for c in range(4)
                            ]
                            fillq += [
                                lambda eb=eb, sc2=sc2, xx=nxt_xt, kk=nxt_kt: emit_proj_chunk(
                                    xx, kk, wk_t, bk_t, eb, sc2
                                )
                                for sc2 in (2, 3)
                                for eb in range(NDB)
                            ]
                            fillq += [
                                lambda eb=eb, sc2=sc2, xx=nxt_xt, qq=nxt_qt: emit_proj_chunk(
                                    xx, qq, wq_t, bq_t, eb, sc2
                                )
                                for sc2 in (2, 3)
                                for eb in range(NDB)
                            ]
                            fillq += [
                                lambda c=c, xx=nxt_xt, vv=nxt_v: emit_v_chunk(xx, vv, c)
                                for c in range(4, 8)
                            ]
                        if sh == 1 and j == 3:
                            emit_norm_half(v_all, vp1, dnp1, 1, 0)
                        if sh == 1 and j >= 4 and b + 1 < B:
                            # staggered: kt/qt sh0 of the next batch
                            w, bias, dst = (
                                (wk_t, bk_t, nxt_kt) if j < 6 else (wq_t, bq_t, nxt_qt)
                            )
                            for eb in range(NDB):
                                emit_proj_chunk(nxt_xt, dst, w, bias, eb, (j % 2))
                        elif sh == 1 and rsv:
                            rsv.pop(0)()
                            if j >= 1 and rsv:
                                rsv.pop(0)()
                        else:
                            pop_fill()
                emit_norm_half(v_all, vp1, dnp1, 1, 1)
                while rsv:
                    rsv.pop(0)()
                if b + 1 < B:
                    pop_fill()

                prev = (b, zt, pt1, vp1)
                if b + 1 < B:
                    xt, kt, qt, v_all = nxt_xt, nxt_kt, nxt_qt, nxt_v

            # ---- tail: last batch's g1 ZT quarters + output projection ----
            pb, pzt, ppt1, pvp1 = prev
            for sq in range(NSC):
                for eh in range(NDB):
                    fillq.append(
                        lambda sq=sq, eh=eh: emit_zt_q(
                            pzt, ppt1, pvp1, 1, sq, eh, on_acc=True
                        )
                    )
                if sq >= 1:
                    fillq += [
                        lambda ob=ob, sq=sq: emit_op_item(pb, pzt, ob, sq - 1, True)
                        for ob in range(NDB)
                    ]
            fillq += [
                lambda ob=ob: emit_op_item(pb, pzt, ob, NSC - 1, True)
                for ob in range(NDB)
            ]
            while fq[0] < len(fillq):
                fillq[fq[0]]()
                fq[0] += 1

    nc.compile()
    return nc


_NC = None


def _get_nc():
    global _NC
    if _NC is None:
        _NC = _build()
    return _NC


def _bf(a):
    return np.ascontiguousarray(np.asarray(a, np.float32)).astype(ml_dtypes.bfloat16)


def _make_in_maps(x, Wq, bq, Wk, bk, Wv, bv, Wo, bo):
    x = np.asarray(x, np.float32)
    scale = np.float32(1.0 / np.sqrt(D))
    xT = _bf(x.transpose(0, 2, 1))
    in_maps = []
    for h in range(H):
        bvh = np.asarray(bv, np.float32)[h]
        m = {
            "xT": xT,
            "wqT": _bf(np.asarray(Wq, np.float32)[h].T * scale),
            "wkT": _bf(np.asarray(Wk, np.float32)[h].T),
            "wvT": _bf(np.asarray(Wv, np.float32)[h].T * np.float32(VC)),
            "woT": _bf(
                np.asarray(Wo, np.float32)[:, h * D : (h + 1) * D].T
                * np.float32(1.0 / VC)
            ),
            "bqc": np.ascontiguousarray(
                (np.asarray(bq, np.float32)[h] * scale).reshape(D, 1)
            ),
            "bkc": np.ascontiguousarray(np.asarray(bk, np.float32)[h].reshape(D, 1)),
            "bvb": np.ascontiguousarray(
                np.broadcast_to(np.tile(bvh * np.float32(VC), 2), (P, 2 * D))
            ).astype(np.float32),
            "boc": np.ascontiguousarray(
                (
                    np.asarray(bo, np.float32) if h == 0 else np.zeros(D, np.float32)
                ).reshape(D, 1)
            ),
            "ebc": np.full((P, 1), -np.log(PBIAS), np.float32),
        }
        in_maps.append(m)
    return in_maps


def kernel(x, Wq, bq, Wk, bk, Wv, bv, Wo, bo, _trace=False, _trace_kwargs=None):
    in_maps = _make_in_maps(x, Wq, bq, Wk, bk, Wv, bv, Wo, bo)
    nc = _get_nc()
    kw = {}
    if _trace:
        kw = dict(trace=True, **(_trace_kwargs or {}))
    br = run_bass_kernel_spmd(nc, in_maps, core_ids=list(range(N_CORES)), **kw)
    acc = np.zeros((B, D, S), np.float32)
    for r in br.results:
        acc += r["outT"]
    out = np.ascontiguousarray(acc.transpose(0, 2, 1))
    if _trace:
        kernel.last_results = br
    return out

